# revision 1
# baseline (speedup 1.0000x reference)
"""Trainium2 Bass kernel for nn_Gtu (gated Toeplitz unit / TNN GTU layer).

  uv = silu(x @ W_uv); u, v = split(uv)
  t  = RPE-MLP(arange(n)) * gamma^k          (per-channel causal Toeplitz coefs)
  o  = causal_conv(t, v)                     (per channel, via length-8192 FFT)
  out = (u * o) @ W_o

8 cores = (batch 0..3) x (d1-half 0..1). Each core handles its batch and 512
channels end-to-end plus the partial output projection; the host sums the two
partials per batch (row-split of W_o) and concatenates batches.

FFT: four-step matmul factorization, L = 8192 = 128 x 64:
  n = n1*64 + n2 (n1 in [0,128) contracted; input support n1 < 64)
  k = k1 + 128*k2
Forward stage A is computed "stationary-swapped": the data chunk [64 n1,
128 (s,n2)] is the PE-stationary operand and the 128-point DFT matrix the
moving one, so the result lands directly in [(s,n2), k1] layout and the
per-tau transpose of the old scheme disappears. t and v share one [128,1024]
tile (t = cols 0:512, v = 512:1024) so stage A/B weights load once for both.
Stage B uses the plain G matrices for both sides; e1/f (= T_re +- T_im) and
their reversed variants are formed on the DVE afterwards. All elementwise
work runs on bf16 SBUF operands (DVE 2x mode); PSUM is drained by ScalarE.
Spectral multiply packs 2 real channels per complex lane:
  S4 = (e1 + rev f) + i(rev f - f);  D4 = (f + rev e1) + i(e1 - rev e1)
  Q4 = Z*S4 + conj(rev Z)*D4
Inverse mirrors forward (transpose kept there); 1/(4L) folded into fi.
"""

import os

import numpy as np
import ml_dtypes

import concourse.bass as bass
import concourse.tile as tile
import concourse.mybir as mybir
from concourse import bacc
from concourse.bass_utils import run_bass_kernel_spmd

F32 = mybir.dt.float32
F32R = mybir.dt.float32r
BF16 = mybir.dt.bfloat16
AF = mybir.ActivationFunctionType
ALU = mybir.AluOpType
AXX = mybir.AxisListType.X

B, N, D = 4, 4096, 1024
D1 = 1024
H = 512
L = 8192
FEAT = 32
RPE_LAYERS = 3
LOWER = 0.99
LN_EPS = 1e-5
NQUAD = 32

_NP_BF16 = ml_dtypes.bfloat16


def _host_consts():
    c = {}
    bf = lambda a: np.ascontiguousarray(a, dtype=_NP_BF16)
    f32 = lambda a: np.ascontiguousarray(a, dtype=np.float32)

    n1 = np.arange(64)[:, None]
    k1 = np.arange(128)[None, :]
    w = np.exp(-2j * np.pi * n1 * k1 / 128.0)
    c["fa_re"], c["fa_im"], c["fa_imn"] = bf(w.real), bf(w.imag), bf(-w.imag)
    # paired moving operands for stage A: one matmul makes (re|im) at once
    c["fa_pa"] = bf(np.concatenate([w.real, w.imag], axis=1))       # [64,256]
    c["fa_pb"] = bf(np.concatenate([-w.imag, w.real], axis=1))

    # twiddle in [(s,n2), (q,k1)] layout: value exp(-2i pi n2 k1 / L),
    # rows (s,n2) = n2 twice, cols tiled 8x over the (side,q) chunks
    n2c = np.arange(64)[:, None]
    k1c = np.arange(128)[None, :]
    blk = np.exp(-2j * np.pi * n2c * k1c / float(L))
    rows = np.vstack([blk, blk])
    twt = np.tile(rows, (1, 8))
    c["twt_re"], c["twt_im"] = bf(twt.real), bf(twt.imag)
    # interleaved (re|im per 128-col chunk) twiddles for the paired layout
    tw_ra = np.concatenate([rows.real, rows.imag], axis=1)   # [128, 256]
    tw_rb = np.concatenate([rows.imag, rows.real], axis=1)
    c["twt_ra"] = bf(np.tile(tw_ra, (1, 8)))                 # [128, 2048]
    c["twt_rb"] = bf(np.tile(tw_rb, (1, 8)))

    n2 = np.arange(64)[:, None]
    k2 = np.arange(64)[None, :]
    g = np.exp(-2j * np.pi * n2 * k2 / 64.0)
    gb = np.zeros((128, 128), np.complex128)
    gb[:64, :64] = g
    gb[64:, 64:] = g
    c["g_re"], c["g_im"], c["g_imn"] = bf(gb.real), bf(gb.imag), bf(-gb.imag)
    gi = np.conj(gb)
    c["gi_re"], c["gi_im"], c["gi_imn"] = bf(gi.real), bf(gi.imag), bf(-gi.imag)

    n2r = np.arange(64)[:, None]
    k1r = np.arange(128)[None, :]
    ti = np.exp(+2j * np.pi * n2r * k1r / float(L))
    tit = np.tile(np.concatenate([ti, ti], axis=0), (1, 4))
    c["ti_re"], c["ti_im"] = bf(tit.real), bf(tit.imag)

    k1f = np.arange(128)[:, None]
    n1f = np.arange(64)[None, :]
    fi = np.exp(+2j * np.pi * k1f * n1f / 128.0) / (4.0 * L)
    c["fi_re"], c["fi_im"], c["fi_imn"] = bf(fi.real), bf(fi.imag), bf(-fi.imag)

    # reversal stage-B variants: Zrev[k2'] uses G columns 63-k2' (main) and
    # (64-k2')%64 (the k1=0 column)
    def blockdiag(m):
        out = np.zeros((128, 128), np.complex128)
        out[:64, :64] = m
        out[64:, 64:] = m
        return out
    n2v = np.arange(64)[:, None]
    k2v = np.arange(64)[None, :]
    # rev as a partition permutation applied to the drained spectrum:
    # revZ[(s,k2'),k1'] = Z[(s,63-k2'),128-k1'] (k1'>=1; rev-read handles k1)
    # and revZ[(s,k2'),0] = Z[(s,(64-k2')%64),0]
    perm = np.zeros((64, 64))
    perm[63 - k2v[0], np.arange(64)] = 1.0
    perm0 = np.zeros((64, 64))
    perm0[(64 - np.arange(64)) % 64, np.arange(64)] = 1.0
    c["prev"] = bf(blockdiag(perm).real)
    c["prev0"] = bf(blockdiag(perm0).real)

    c["ident"] = bf(np.eye(128))
    c["ident_f32"] = f32(np.eye(128))

    p = np.arange(128)
    c["idxmat"] = f32(p[:, None] + 128.0 * np.arange(32)[None, :])
    c["pbc"] = f32(np.tile(p[None, :], (128, 1)))
    c["j128"] = f32(np.tile(128.0 * np.arange(32)[None, :], (128, 1)))
    return c


CONSTS = _host_consts()


def build_program(nc):
    x = nc.dram_tensor("x", [N, D], F32, kind="ExternalInput")
    wuv = nc.dram_tensor("wuv", [D, 2 * H], F32, kind="ExternalInput")
    wo = nc.dram_tensor("wo", [H, D], F32, kind="ExternalInput")
    rpeo = nc.dram_tensor("rpeo", [FEAT, H], F32, kind="ExternalInput")
    dg = nc.dram_tensor("dg", [1, H], F32, kind="ExternalInput")
    rpe_in = nc.dram_tensor("rpe_in", [1, FEAT], F32, kind="ExternalInput")
    rpe_hid = nc.dram_tensor("rpe_hid", [RPE_LAYERS, FEAT, FEAT], F32,
                             kind="ExternalInput")
    ln_g = nc.dram_tensor("ln_g", [RPE_LAYERS, FEAT], F32, kind="ExternalInput")
    ln_b = nc.dram_tensor("ln_b", [RPE_LAYERS, FEAT], F32, kind="ExternalInput")

    cds = {}
    for name, arr in CONSTS.items():
        dt = BF16 if arr.dtype == _NP_BF16 else F32
        cds[name] = nc.dram_tensor(name, list(arr.shape), dt, kind="ExternalInput")

    t_bf = nc.dram_tensor("t_bf", [H, N], BF16, kind="Internal")
    v_bf = nc.dram_tensor("v_bf", [H, N], BF16, kind="Internal")
    u_bf = nc.dram_tensor("u_bf", [H, N], BF16, kind="Internal")
    o_bf = nc.dram_tensor("o_bf", [H, N], BF16, kind="Internal")
    lng_dram = nc.dram_tensor("lng_dram", [1, H], F32, kind="Internal")
    out = nc.dram_tensor("out", [D, N], BF16, kind="ExternalOutput")

    ap = lambda t, off, pattern: bass.AP(tensor=t, offset=off, ap=pattern)

    with tile.TileContext(nc) as tc:
        with tc.tile_pool(name="consts", bufs=1) as cp:
            cs = {}
            for name, arr in CONSTS.items():
                dt = BF16 if arr.dtype == _NP_BF16 else F32
                ct = cp.tile(list(arr.shape), dt, tag=f"c_{name}")
                nc.sync.dma_start(out=ct, in_=cds[name][:, :])
                cs[name] = ct
            ident_r = cp.tile([128, 128], F32R, tag="ident_r")
            nc.gpsimd.dma_start(out=ident_r, in_=cds["ident_f32"][:, :])

            w_in_bc = cp.tile([128, FEAT], F32, tag="w_in_bc")
            nc.sync.dma_start(out=w_in_bc, in_=ap(rpe_in, 0, [[0, 128], [1, FEAT]]))
            lng_bc, lnb_bc = [], []
            for l in range(RPE_LAYERS):
                g_t = cp.tile([128, FEAT], F32, tag=f"lng{l}")
                b_t = cp.tile([128, FEAT], F32, tag=f"lnb{l}")
                nc.sync.dma_start(out=g_t, in_=ap(ln_g, l * FEAT, [[0, 128], [1, FEAT]]))
                nc.sync.dma_start(out=b_t, in_=ap(ln_b, l * FEAT, [[0, 128], [1, FEAT]]))
                lng_bc.append(g_t)
                lnb_bc.append(b_t)

            whid = []
            for l in range(RPE_LAYERS):
                wt4 = cp.tile([128, 128], F32, tag=f"whid{l}")
                nc.vector.memset(wt4, 0.0)
                for j in range(4):
                    nc.sync.dma_start(
                        out=wt4[32 * j:32 * j + 32, 32 * j:32 * j + 32],
                        in_=rpe_hid[l, :, :])
                whid.append(wt4)

            eps_t = cp.tile([128, 1], F32, tag="eps_t")
            nc.vector.memset(eps_t, LN_EPS)
            rpeo_sb = cp.tile([128, H], F32, tag="rpeo_sb")
            for j in range(4):
                nc.sync.dma_start(out=rpeo_sb[32 * j:32 * j + 32, :], in_=rpeo[:, :])

            # decay -> lngam_col [128, 4]  (lngam_col[cp, cb] = ln gamma_{128cb+cp})
            with tc.tile_pool(name="dk", bufs=1) as dk:
                dg_sb = dk.tile([1, H], F32, tag="dg")
                nc.sync.dma_start(out=dg_sb, in_=dg[:, :])
                sig = dk.tile([1, H], F32, tag="sig")
                nc.scalar.activation(sig, dg_sb, AF.Sigmoid)
                gam = dk.tile([1, H], F32, tag="gam")
                nc.vector.tensor_scalar(gam, sig, 1.0 - LOWER, LOWER,
                                        ALU.mult, ALU.add)
                lngr = dk.tile([1, H], F32, tag="lngr")
                nc.scalar.activation(lngr, gam, AF.Ln)
                nc.sync.dma_start(out=lng_dram[:, :], in_=lngr)
            lngam_col = cp.tile([128, 4], F32, tag="lngam_col")
            nc.sync.dma_start(out=lngam_col,
                              in_=ap(lng_dram, 0, [[1, 128], [128, 4]]))

            # decay matrices: d0[cb][ch, s] = gamma_ch^s; rj[cb][ch, j] =
            # gamma_ch^(128 j) -- replaces a scalar Exp per (j, cb) block
            d0s, rjs = [], []
            for cb in range(4):
                d0 = cp.tile([128, 128], BF16, tag=f"d0_{cb}")
                nc.scalar.activation(d0, cs["pbc"], AF.Exp,
                                     scale=lngam_col[:, cb:cb + 1])
                rj = cp.tile([128, 32], F32, tag=f"rj_{cb}")
                nc.scalar.activation(rj, cs["j128"], AF.Exp,
                                     scale=lngam_col[:, cb:cb + 1])
                d0s.append(d0)
                rjs.append(rj)

            # =====================================================
            # Phase A: RPE MLP -> t_bf (channel-major bf16)
            # =====================================================
            idxm = cs["idxmat"]
            with tc.tile_pool(name="rpe", bufs=2) as rp, \
                 tc.tile_pool(name="rpe_ps", bufs=2, space="PSUM") as rps:
                for grp in range(8):
                    h_sm = rp.tile([128, 4, FEAT], F32, tag="h_sm")
                    for jj in range(4):
                        j = 4 * grp + jj
                        nc.scalar.activation(h_sm[:, jj, :], w_in_bc, AF.Silu,
                                             scale=idxm[:, j:j + 1])
                    h_fm = None
                    for l in range(RPE_LAYERS):
                        mu = rp.tile([128, 4], F32, tag="mu")
                        nc.vector.tensor_reduce(mu, h_sm, AXX, ALU.add)
                        nc.vector.tensor_scalar_mul(mu, mu, 1.0 / FEAT)
                        hc = rp.tile([128, 4, FEAT], F32, tag="hc")
                        nc.vector.tensor_tensor(
                            hc, h_sm, mu[:, :, None].to_broadcast((128, 4, FEAT)),
                            ALU.subtract)
                        sq = rp.tile([128, 4, FEAT], F32, tag="sq")
                        nc.scalar.activation(sq, hc, AF.Square)
                        var = rp.tile([128, 4], F32, tag="var")
                        nc.vector.tensor_reduce(var, sq, AXX, ALU.add)
                        rstd = rp.tile([128, 4], F32, tag="rstd")
                        nc.scalar.activation(rstd, var, AF.Sqrt,
                                             scale=1.0 / FEAT, bias=eps_t)
                        nc.vector.reciprocal(rstd, rstd)
                        hn = rp.tile([128, 4, FEAT], F32, tag="hn")
                        nc.vector.tensor_tensor(
                            hn, hc, rstd[:, :, None].to_broadcast((128, 4, FEAT)),
                            ALU.mult)
                        gb_ = lng_bc[l][:, None, :].to_broadcast((128, 4, FEAT))
                        bb_ = lnb_bc[l][:, None, :].to_broadcast((128, 4, FEAT))
                        hs = rp.tile([128, 4, FEAT], F32, tag="hs")
                        nc.vector.tensor_tensor(hs, hn, gb_, ALU.mult)
                        nc.vector.tensor_tensor(hs, hs, bb_, ALU.add)
                        pt = rps.tile([128, 128], F32, tag="tp")
                        nc.tensor.transpose(
                            pt, hs.rearrange("p a b -> p (a b)"), cs["ident_f32"])
                        ln_fm = rp.tile([128, 128], F32, tag="ln_fm")
                        nc.vector.tensor_copy(out=ln_fm, in_=pt)
                        hp = rps.tile([128, 128], F32, tag="mm")
                        nc.tensor.matmul(hp, whid[l], ln_fm, start=True, stop=True)
                        h_fm = rp.tile([128, 128], F32, tag="h_fm")
                        nc.scalar.activation(h_fm, hp, AF.Silu)
                        if l < RPE_LAYERS - 1:
                            pt2 = rps.tile([128, 128], F32, tag="tp")
                            nc.tensor.transpose(pt2, h_fm, cs["ident_f32"])
                            nc.vector.tensor_copy(
                                out=h_sm.rearrange("p a b -> p (a b)"), in_=pt2)
                    for jj in range(4):
                        j = 4 * grp + jj
                        for cb in range(4):
                            tp = rps.tile([128, 128], F32, tag="tmm")
                            nc.tensor.matmul(
                                tp,
                                rpeo_sb[32 * jj:32 * jj + 32,
                                        128 * cb:128 * cb + 128],
                                h_fm[32 * jj:32 * jj + 32, :],
                                start=True, stop=True,
                                tile_position=(32 * jj, 0))
                            tsc = rp.tile([128, 128], BF16, tag="tsc")
                            nc.vector.tensor_scalar_mul(
                                tsc, tp, rjs[cb][:, j:j + 1])
                            tt = rp.tile([128, 128], BF16, tag="t_out")
                            nc.vector.tensor_tensor(tt, tsc, d0s[cb], ALU.mult)
                            nc.sync.dma_start(
                                out=ap(t_bf, 128 * cb * N + 128 * j,
                                       [[N, 128], [1, 128]]),
                                in_=tt)

            # =====================================================
            # Phase B: uv projection (f32r) + silu -> u_bf, v_bf
            # =====================================================
            with tc.tile_pool(name="pb_w", bufs=1) as wbp, \
                 tc.tile_pool(name="pb", bufs=2) as pb, \
                 tc.tile_pool(name="pb_ps", bufs=2, space="PSUM") as pps:
                wuv_f = wbp.tile([128, 8, 2 * H], F32, tag="wuv_f")
                nc.gpsimd.dma_start(
                    out=wuv_f,
                    in_=ap(wuv, 0, [[2 * H, 128], [128 * 2 * H, 8], [1, 2 * H]]))
                wuv_sb = wbp.tile([128, 8, 2 * H], BF16, tag="wuv_sb")
                nc.vector.tensor_copy(out=wuv_sb, in_=wuv_f)
                for stg in range(8):
                    xT = pb.tile([128, 8, 512], BF16, tag="xT")
                    for st4 in range(4):
                        st = 4 * stg + st4
                        xt = pb.tile([128, D], F32R, tag="x_in")
                        nc.gpsimd.dma_start(out=xt,
                                            in_=x[128 * st:128 * st + 128, :])
                        for k in range(8):
                            ptx = pps.tile([128, 128], F32R, tag="x_tp")
                            nc.tensor.transpose(
                                ptx, xt[:, 128 * k:128 * k + 128], ident_r)
                            nc.scalar.activation(
                                xT[:, k, 128 * st4:128 * st4 + 128], ptx, AF.Copy)
                    for cblk in range(8):
                        pu = pps.tile([128, 512], F32, tag="uv_mm")
                        for k in range(8):
                            nc.tensor.matmul(
                                pu, wuv_sb[:, k, 128 * cblk:128 * cblk + 128],
                                xT[:, k, :], start=(k == 0), stop=(k == 7))
                        uv_sb = pb.tile([128, 512], BF16, tag="uv_out")
                        nc.scalar.activation(uv_sb, pu, AF.Silu)
                        dst = u_bf if cblk < 4 else v_bf
                        cbase = (cblk % 4) * 128
                        nc.sync.dma_start(
                            out=ap(dst, cbase * N + 512 * stg,
                                   [[N, 128], [1, 512]]),
                            in_=uv_sb)

            # =====================================================
            # Phase C: FFT conv, one quad (16 channels) at a time.
            # t occupies free cols 0:512, v cols 512:1024 of shared tiles.
            # =====================================================
            with tc.tile_pool(name="pc", bufs=2) as pc, \
                 tc.tile_pool(name="pc_sp", bufs=2) as sp, \
                 tc.tile_pool(name="pc_ps", bufs=1, space="PSUM") as ps:

                def mm(psum_out, lhsT, rhs, start, stop):
                    nc.tensor.matmul(psum_out, lhsT, rhs, start=start, stop=stop,
                                     skip_group_check=True)

                def front(q4):
                    """gather + stage A + twiddle + stage B (straight & rev).
                    PSUM p0-p3 only. Returns drained spectra tiles."""
                    rr = pc.tile([64, 1024], BF16, tag="rr", name="rr")
                    ri = pc.tile([64, 1024], BF16, tag="ri", name="ri")
                    base = 16 * q4 * N
                    pat = [[64, 64], [4 * N, 4], [2 * N, 2], [1, 64]]
                    nc.sync.dma_start(out=rr[:, 0:512], in_=ap(t_bf, base, pat))
                    nc.sync.dma_start(out=ri[:, 0:512], in_=ap(t_bf, base + N, pat))
                    nc.sync.dma_start(out=rr[:, 512:1024], in_=ap(v_bf, base, pat))
                    nc.sync.dma_start(out=ri[:, 512:1024], in_=ap(v_bf, base + N, pat))

                    # stage A (stationary-swapped, paired re|im moving operand)
                    pa = [ps.tile([128, 512], F32, tag=f"p{i}", name=f"pa{i}")
                          for i in range(4)]
                    for c in range(8):
                        dst = pa[c // 2][:, 256 * (c % 2):256 * (c % 2) + 256]
                        mm(dst, rr[:, 128 * c:128 * c + 128], cs["fa_pa"], True, False)
                        mm(dst, ri[:, 128 * c:128 * c + 128], cs["fa_pb"], False, True)
                    # drain verbatim (interleaved re|im layout), 4 ACT copies
                    asb = pc.tile([128, 2048], BF16, tag="asb", name="asb")
                    for i in range(4):
                        nc.scalar.activation(asb[:, 512 * i:512 * i + 512],
                                             pa[i], AF.Copy)

                    # twiddle in interleaved layout, de-interleave on the DVE
                    m1p = pc.tile([128, 2048], BF16, tag="m1p", name="m1p")
                    m2p = pc.tile([128, 2048], BF16, tag="m2p", name="m2p")
                    bt_re = pc.tile([128, 1024], BF16, tag="btre", name="btre")
                    bt_im = pc.tile([128, 1024], BF16, tag="btim", name="btim")
                    nc.vector.tensor_tensor(m1p, asb, cs["twt_ra"], ALU.mult)
                    nc.vector.tensor_tensor(m2p, asb, cs["twt_rb"], ALU.mult)
                    po_ = m1p.ap[0][0]
                    re_sl = lambda t_: ap(t_.tensor, t_.offset,
                                          [[po_, 128], [256, 8], [1, 128]])
                    im_sl = lambda t_: ap(t_.tensor, t_.offset + 128,
                                          [[po_, 128], [256, 8], [1, 128]])
                    bt_re3 = bt_re[:, bass.ts(0, 1024)].rearrange(
                        "p (c k) -> p c k", c=8)
                    bt_im3 = bt_im[:, bass.ts(0, 1024)].rearrange(
                        "p (c k) -> p c k", c=8)
                    nc.vector.tensor_tensor(bt_re3, re_sl(m1p), im_sl(m1p),
                                            ALU.subtract)
                    nc.vector.tensor_tensor(bt_im3, re_sl(m2p), im_sl(m2p),
                                            ALU.add)

                    # stage B straight -> p0-p3 (reused after asb drain)
                    pz = [ps.tile([128, 512], F32, tag=f"p{i}", name=f"pz{i}")
                          for i in range(4)]
                    # pz[0]=T_re pz[1]=Z_re pz[2]=T_im pz[3]=Z_im
                    bre_t, bre_v = bt_re[:, 0:512], bt_re[:, 512:1024]
                    bim_t, bim_v = bt_im[:, 0:512], bt_im[:, 512:1024]
                    mm(pz[2], cs["g_im"], bre_t, True, False)
                    mm(pz[3], cs["g_im"], bre_v, True, False)
                    mm(pz[0], cs["g_re"], bre_t, True, False)
                    mm(pz[1], cs["g_re"], bre_v, True, False)
                    mm(pz[2], cs["g_re"], bim_t, False, True)
                    mm(pz[3], cs["g_re"], bim_v, False, True)
                    mm(pz[0], cs["g_imn"], bim_t, False, True)
                    mm(pz[1], cs["g_imn"], bim_v, False, True)
                    tt_re = sp.tile([128, 1024], BF16, tag="tt_re", name="tt_re")
                    tt_im = sp.tile([128, 1024], BF16, tag="tt_im", name="tt_im")
                    vz_re = sp.tile([128, 1024], BF16, tag="vz_re", name="vz_re")
                    vz_im = sp.tile([128, 1024], BF16, tag="vz_im", name="vz_im")
                    nc.scalar.activation(tt_re[:, 0:512], pz[0], AF.Copy)
                    nc.scalar.activation(vz_re[:, 0:512], pz[1], AF.Copy)
                    nc.scalar.activation(tt_im[:, 0:512], pz[2], AF.Copy)
                    nc.scalar.activation(vz_im[:, 0:512], pz[3], AF.Copy)

                    # rev via permutation matmul on the drained spectra
                    def rev_rhs(t_):
                        return ap(t_.tensor, t_.offset + 127,
                                  [[t_.ap[0][0], 128], [128, 4], [-1, 127]])

                    def col0_rhs(t_):
                        return ap(t_.tensor, t_.offset,
                                  [[t_.ap[0][0], 128], [128, 4]])

                    pr = [ps.tile([128, 512], F32, tag=f"p{i}", name=f"pr{i}")
                          for i in range(4)]
                    # pr[0]=Tr_re pr[1]=Zr_re pr[2]=Tr_im pr[3]=Zr_im
                    def rslice(t_, a, b):
                        return t_[:, bass.ts(0, 512)].rearrange(
                            "p (q k) -> p q k", q=4)[:, :, a:b]
                    mm(rslice(pr[0], 1, 128), cs["prev"], rev_rhs(tt_re[:, 0:512]), True, True)
                    mm(rslice(pr[1], 1, 128), cs["prev"], rev_rhs(vz_re[:, 0:512]), True, True)
                    mm(rslice(pr[2], 1, 128), cs["prev"], rev_rhs(tt_im[:, 0:512]), True, True)
                    mm(rslice(pr[3], 1, 128), cs["prev"], rev_rhs(vz_im[:, 0:512]), True, True)
                    mm(rslice(pr[0], 0, 1), cs["prev0"], col0_rhs(tt_re[:, 0:512]), True, True)
                    mm(rslice(pr[1], 0, 1), cs["prev0"], col0_rhs(vz_re[:, 0:512]), True, True)
                    mm(rslice(pr[2], 0, 1), cs["prev0"], col0_rhs(tt_im[:, 0:512]), True, True)
                    mm(rslice(pr[3], 0, 1), cs["prev0"], col0_rhs(vz_im[:, 0:512]), True, True)
                    nc.scalar.activation(tt_re[:, 512:1024], pr[0], AF.Copy)
                    nc.scalar.activation(vz_re[:, 512:1024], pr[1], AF.Copy)
                    nc.scalar.activation(tt_im[:, 512:1024], pr[2], AF.Copy)
                    nc.scalar.activation(vz_im[:, 512:1024], pr[3], AF.Copy)
                    return dict(tt_re=tt_re, tt_im=tt_im, vz_re=vz_re,
                                vz_im=vz_im, base=base, pat=pat)

                def back(st):
                    """combos + spectral multiply + inverse. PSUM p4-p7."""
                    tt_re, tt_im = st["tt_re"], st["tt_im"]
                    vz_re, vz_im = st["vz_re"], st["vz_im"]
                    e1p = sp.tile([128, 1024], BF16, tag="e1p", name="e1p")
                    ffp = sp.tile([128, 1024], BF16, tag="ffp", name="ffp")
                    nc.vector.tensor_tensor(e1p, tt_re, tt_im, ALU.add)
                    nc.vector.tensor_tensor(ffp, tt_re, tt_im, ALU.subtract)
                    s4re = sp.tile([128, 512], BF16, tag="s4re", name="s4re")
                    s4im = sp.tile([128, 512], BF16, tag="s4im", name="s4im")
                    d4re = sp.tile([128, 512], BF16, tag="d4re", name="d4re")
                    d4im = sp.tile([128, 512], BF16, tag="d4im", name="d4im")
                    nc.vector.tensor_tensor(s4re, e1p[:, 0:512], e1p[:, 512:1024], ALU.add)
                    nc.vector.tensor_tensor(s4im, ffp[:, 512:1024], ffp[:, 0:512], ALU.subtract)
                    nc.gpsimd.tensor_tensor(d4re, ffp[:, 0:512], ffp[:, 512:1024], ALU.add)
                    nc.gpsimd.tensor_tensor(d4im, e1p[:, 0:512], e1p[:, 512:1024], ALU.subtract)

                    zv_re, zv_im = vz_re[:, 0:512], vz_im[:, 0:512]
                    zr_re, zr_im = vz_re[:, 512:1024], vz_im[:, 512:1024]
                    q_re = sp.tile([128, 512], BF16, tag="q_re", name="q_re")
                    q_im = sp.tile([128, 512], BF16, tag="q_im", name="q_im")
                    a1 = pc.tile([128, 512], BF16, tag="qa1", name="qa1")
                    a2 = pc.tile([128, 512], BF16, tag="qa2", name="qa2")
                    a3 = pc.tile([128, 512], BF16, tag="qa3", name="qa3")
                    a4 = pc.tile([128, 512], BF16, tag="qa4", name="qa4")
                    nc.vector.tensor_tensor(a1, zv_re, s4re, ALU.mult)
                    nc.vector.tensor_tensor(a2, zv_im, s4im, ALU.mult)
                    nc.vector.tensor_tensor(a3, zr_re, d4re, ALU.mult)
                    nc.vector.tensor_tensor(a4, zr_im, d4im, ALU.mult)
                    nc.vector.tensor_tensor(a1, a1, a2, ALU.subtract)
                    nc.vector.tensor_tensor(a3, a3, a4, ALU.add)
                    nc.vector.tensor_tensor(q_re, a1, a3, ALU.add)
                    b1 = pc.tile([128, 512], BF16, tag="qb1", name="qb1")
                    b2 = pc.tile([128, 512], BF16, tag="qb2", name="qb2")
                    b3 = pc.tile([128, 512], BF16, tag="qb3", name="qb3")
                    b4 = pc.tile([128, 512], BF16, tag="qb4", name="qb4")
                    nc.gpsimd.tensor_tensor(b1, zv_re, s4im, ALU.mult)
                    nc.gpsimd.tensor_tensor(b2, zv_im, s4re, ALU.mult)
                    nc.gpsimd.tensor_tensor(b1, b1, b2, ALU.add)
                    nc.vector.tensor_tensor(b3, zr_re, d4im, ALU.mult)
                    nc.vector.tensor_tensor(b4, zr_im, d4re, ALU.mult)
                    nc.vector.tensor_tensor(b3, b3, b4, ALU.subtract)
                    nc.vector.tensor_tensor(q_im, b1, b3, ALU.add)

                    pc_re = ps.tile([128, 512], F32, tag="p4", name="pc_re")
                    pc_im = ps.tile([128, 512], F32, tag="p5", name="pc_im")
                    mm(pc_im, cs["gi_im"], q_re, True, False)
                    mm(pc_re, cs["gi_re"], q_re, True, False)
                    mm(pc_im, cs["gi_re"], q_im, False, True)
                    mm(pc_re, cs["gi_imn"], q_im, False, True)
                    csb_re = pc.tile([128, 512], BF16, tag="csb_re", name="csb_re")
                    csb_im = pc.tile([128, 512], BF16, tag="csb_im", name="csb_im")
                    nc.scalar.activation(csb_re, pc_re, AF.Copy)
                    nc.scalar.activation(csb_im, pc_im, AF.Copy)
                    i1 = pc.tile([128, 512], BF16, tag="i1", name="i1")
                    i2 = pc.tile([128, 512], BF16, tag="i2", name="i2")
                    i3 = pc.tile([128, 512], BF16, tag="i3", name="i3")
                    i4 = pc.tile([128, 512], BF16, tag="i4", name="i4")
                    ct_re = pc.tile([128, 512], BF16, tag="ct_re", name="ct_re")
                    ct_im = pc.tile([128, 512], BF16, tag="ct_im", name="ct_im")
                    nc.vector.tensor_tensor(i1, csb_re, cs["ti_re"], ALU.mult)
                    nc.vector.tensor_tensor(i2, csb_im, cs["ti_im"], ALU.mult)
                    nc.vector.tensor_tensor(i3, csb_re, cs["ti_im"], ALU.mult)
                    nc.vector.tensor_tensor(i4, csb_im, cs["ti_re"], ALU.mult)
                    nc.gpsimd.tensor_tensor(ct_re, i1, i2, ALU.subtract)
                    nc.gpsimd.tensor_tensor(ct_im, i3, i4, ALU.add)
                    tp_re = ps.tile([128, 512], BF16, tag="p6", name="tp_re")
                    tp_im = ps.tile([128, 512], BF16, tag="p7", name="tp_im")
                    for tau in range(4):
                        nc.tensor.transpose(
                            tp_re[:, 128 * tau:128 * tau + 128],
                            ct_re[:, 128 * tau:128 * tau + 128], cs["ident"])
                        nc.tensor.transpose(
                            tp_im[:, 128 * tau:128 * tau + 128],
                            ct_im[:, 128 * tau:128 * tau + 128], cs["ident"])
                    ctt_re = pc.tile([128, 512], BF16, tag="ctt_re", name="ctt_re")
                    ctt_im = pc.tile([128, 512], BF16, tag="ctt_im", name="ctt_im")
                    nc.vector.tensor_copy(out=ctt_re, in_=tp_re)
                    nc.vector.tensor_copy(out=ctt_im, in_=tp_im)
                    pO_re = ps.tile([64, 512], F32, tag="p4", name="pO_re")
                    pO_im = ps.tile([64, 512], F32, tag="p5", name="pO_im")
                    mm(pO_im, cs["fi_im"], ctt_re, True, False)
                    mm(pO_re, cs["fi_re"], ctt_re, True, False)
                    mm(pO_im, cs["fi_re"], ctt_im, False, True)
                    mm(pO_re, cs["fi_imn"], ctt_im, False, True)
                    o_re = pc.tile([64, 512], BF16, tag="o_re", name="o_re")
                    o_im = pc.tile([64, 512], BF16, tag="o_im", name="o_im")
                    nc.scalar.activation(o_re, pO_re, AF.Copy)
                    nc.scalar.activation(o_im, pO_im, AF.Copy)
                    nc.sync.dma_start(out=ap(o_bf, st["base"], st["pat"]), in_=o_re)
                    nc.sync.dma_start(out=ap(o_bf, st["base"] + N, st["pat"]), in_=o_im)

                pend = None
                for q4 in range(NQUAD):
                    cur = front(q4)
                    if pend is not None:
                        back(pend)
                    pend = cur
                back(pend)

            # =====================================================
            # Phase D: gate + output projection (f32r partials)
            # =====================================================
            with tc.tile_pool(name="pd_w", bufs=1) as wdp, \
                 tc.tile_pool(name="pd", bufs=2) as pd, \
                 tc.tile_pool(name="pd_ps", bufs=2, space="PSUM") as dps:
                wo_f = wdp.tile([128, 4, D], F32, tag="wo_f")
                nc.gpsimd.dma_start(
                    out=wo_f, in_=ap(wo, 0, [[D, 128], [128 * D, 4], [1, D]]))
                wo_sb = wdp.tile([128, 4, D], BF16, tag="wo_sb")
                nc.vector.tensor_copy(out=wo_sb, in_=wo_f)
                for sb in range(8):
                    gts = []
                    for cb in range(4):
                        ut = pd.tile([128, 512], BF16, tag=f"g_u{cb}")
                        ot = pd.tile([128, 512], BF16, tag=f"g_o{cb}")
                        nc.sync.dma_start(
                            out=ut, in_=ap(u_bf, 128 * cb * N + 512 * sb,
                                           [[N, 128], [1, 512]]))
                        nc.sync.dma_start(
                            out=ot, in_=ap(o_bf, 128 * cb * N + 512 * sb,
                                           [[N, 128], [1, 512]]))
                        gt = pd.tile([128, 512], BF16, tag=f"g_g{cb}")
                        nc.vector.tensor_tensor(gt, ut, ot, ALU.mult)
                        gts.append(gt)
                    for ocblk in range(8):
                        po = dps.tile([128, 512], F32, tag="out_mm")
                        for cb in range(4):
                            nc.tensor.matmul(
                                po, wo_sb[:, cb, 128 * ocblk:128 * ocblk + 128],
                                gts[cb], start=(cb == 0), stop=(cb == 3))
                        os_ = pd.tile([128, 512], BF16, tag="out_sb")
                        nc.scalar.activation(os_, po, AF.Copy)
                        nc.sync.dma_start(
                            out=ap(out, 128 * ocblk * N + 512 * sb,
                                   [[N, 128], [1, 512]]),
                            in_=os_)
    return nc


_PROGRAM_CACHE = {}
LAST_RESULTS = []


def _get_program():
    if "nc" not in _PROGRAM_CACHE:
        nc = bacc.Bacc("TRN2", target_bir_lowering=False)
        build_program(nc)
        nc.compile()
        _PROGRAM_CACHE["nc"] = nc
    return _PROGRAM_CACHE["nc"]


def kernel(x, W_uv, W_o, rpe_in_w, rpe_hid_w, rpe_ln_g, rpe_ln_b, rpe_out_w,
           decay_gamma):
    x = np.asarray(x, np.float32)
    W_uv = np.asarray(W_uv, np.float32)
    W_o = np.asarray(W_o, np.float32)

    nc = _get_program()

    shared = dict(CONSTS)
    shared["rpe_in"] = np.ascontiguousarray(rpe_in_w, np.float32)
    shared["rpe_hid"] = np.ascontiguousarray(rpe_hid_w, np.float32)
    shared["ln_g"] = np.ascontiguousarray(rpe_ln_g, np.float32)
    shared["ln_b"] = np.ascontiguousarray(rpe_ln_b, np.float32)

    in_maps = []
    for core in range(8):
        b, h = core // 2, core % 2
        c0 = h * H
        m = dict(shared)
        m["x"] = np.ascontiguousarray(x[b])
        m["wuv"] = np.ascontiguousarray(
            np.concatenate([W_uv[:, c0:c0 + H], W_uv[:, D1 + c0:D1 + c0 + H]],
                           axis=1))
        m["wo"] = np.ascontiguousarray(np.asarray(W_o, np.float32)[c0:c0 + H, :])
        m["rpeo"] = np.ascontiguousarray(np.asarray(rpe_out_w, np.float32)[:, c0:c0 + H])
        m["dg"] = np.ascontiguousarray(
            np.asarray(decay_gamma, np.float32)[None, c0:c0 + H])
        in_maps.append(m)

    trace = os.environ.get("KERNEL_TRACE", "0") == "1"
    tkw = {}
    if trace:
        tkw = dict(trace=True,
                   trace_cores=[int(c) for c in os.environ.get(
                       "KERNEL_TRACE_CORES", "0").split(",")])
    res = run_bass_kernel_spmd(nc, in_maps, core_ids=list(range(8)), **tkw)
    LAST_RESULTS.append(res)
    outs = [np.asarray(r["out"], np.float32) for r in res.results]
    final = np.empty((B, N, D), np.float32)
    for b in range(B):
        final[b] = (outs[2 * b] + outs[2 * b + 1]).T
    return final



# revision 11
# speedup vs baseline: 1.1243x; 1.1243x over previous
"""Trainium2 Bass kernel for nn_Gtu (gated Toeplitz unit / TNN GTU layer).

  uv = silu(x @ W_uv); u, v = split(uv)
  t  = RPE-MLP(arange(n)) * gamma^k          (per-channel causal Toeplitz coefs)
  o  = causal_conv(t, v)                     (per channel, via length-8192 FFT)
  out = (u * o) @ W_o

8 cores = (batch 0..3) x (d1-half 0..1). Each core handles its batch and 512
channels end-to-end plus the partial output projection; the host sums the two
partials per batch (row-split of W_o) and concatenates batches.

FFT: four-step matmul factorization, L = 8192 = 128 x 64:
  n = n1*64 + n2 (n1 in [0,128) contracted; input support n1 < 64)
  k = k1 + 128*k2
Forward stage A is computed "stationary-swapped": the data chunk [64 n1,
128 (s,n2)] is the PE-stationary operand and the 128-point DFT matrix the
moving one, so the result lands directly in [(s,n2), k1] layout and the
per-tau transpose of the old scheme disappears. t and v share one [128,1024]
tile (t = cols 0:512, v = 512:1024) so stage A/B weights load once for both.
Stage B uses the plain G matrices for both sides; e1/f (= T_re +- T_im) and
their reversed variants are formed on the DVE afterwards. All elementwise
work runs on bf16 SBUF operands (DVE 2x mode); PSUM is drained by ScalarE.
Spectral multiply packs 2 real channels per complex lane:
  S4 = (e1 + rev f) + i(rev f - f);  D4 = (f + rev e1) + i(e1 - rev e1)
  Q4 = Z*S4 + conj(rev Z)*D4
Inverse mirrors forward (transpose kept there); 1/(4L) folded into fi.
"""

import os

import numpy as np
import ml_dtypes

import concourse.bass as bass
import concourse.tile as tile
import concourse.mybir as mybir
from concourse import bacc
from concourse.bass_utils import run_bass_kernel_spmd

F32 = mybir.dt.float32
F32R = mybir.dt.float32r
BF16 = mybir.dt.bfloat16
AF = mybir.ActivationFunctionType
ALU = mybir.AluOpType
AXX = mybir.AxisListType.X

B, N, D = 4, 4096, 1024
D1 = 1024
H = 512
L = 8192
FEAT = 32
RPE_LAYERS = 3
LOWER = 0.99
LN_EPS = 1e-5
NQUAD = 32

_NP_BF16 = ml_dtypes.bfloat16


def _host_consts():
    c = {}
    bf = lambda a: np.ascontiguousarray(a, dtype=_NP_BF16)
    f32 = lambda a: np.ascontiguousarray(a, dtype=np.float32)

    n1 = np.arange(64)[:, None]
    k1 = np.arange(128)[None, :]
    w = np.exp(-2j * np.pi * n1 * k1 / 128.0)
    c["fa_re"], c["fa_im"], c["fa_imn"] = bf(w.real), bf(w.imag), bf(-w.imag)
    # paired moving operands for stage A: one matmul makes (re|im) at once
    c["fa_pa"] = bf(np.concatenate([w.real, w.imag], axis=1))       # [64,256]
    c["fa_pb"] = bf(np.concatenate([-w.imag, w.real], axis=1))

    # twiddle in [(s,n2), (q,k1)] layout: value exp(-2i pi n2 k1 / L),
    # rows (s,n2) = n2 twice, cols tiled 8x over the (side,q) chunks
    n2c = np.arange(64)[:, None]
    k1c = np.arange(128)[None, :]
    blk = np.exp(-2j * np.pi * n2c * k1c / float(L))
    rows = np.vstack([blk, blk])
    twt = np.tile(rows, (1, 8))
    c["twt_re"], c["twt_im"] = bf(twt.real), bf(twt.imag)
    # interleaved (re|im per 128-col chunk) twiddles for the paired layout
    tw_ra = np.concatenate([rows.real, rows.imag], axis=1)   # [128, 256]
    tw_rb = np.concatenate([rows.imag, rows.real], axis=1)
    c["twt_ra"] = bf(np.tile(tw_ra, (1, 8)))                 # [128, 2048]
    c["twt_rb"] = bf(np.tile(tw_rb, (1, 8)))

    n2 = np.arange(64)[:, None]
    k2 = np.arange(64)[None, :]
    g = np.exp(-2j * np.pi * n2 * k2 / 64.0)
    gb = np.zeros((128, 128), np.complex128)
    gb[:64, :64] = g
    gb[64:, 64:] = g
    c["g_re"], c["g_im"], c["g_imn"] = bf(gb.real), bf(gb.imag), bf(-gb.imag)
    gi = np.conj(gb)
    c["gi_re"], c["gi_im"], c["gi_imn"] = bf(gi.real), bf(gi.imag), bf(-gi.imag)

    n2r = np.arange(64)[:, None]
    k1r = np.arange(128)[None, :]
    ti = np.exp(+2j * np.pi * n2r * k1r / float(L))
    tit = np.tile(np.concatenate([ti, ti], axis=0), (1, 4))
    c["ti_re"], c["ti_im"] = bf(tit.real), bf(tit.imag)

    k1f = np.arange(128)[:, None]
    n1f = np.arange(64)[None, :]
    fi = np.exp(+2j * np.pi * k1f * n1f / 128.0) / (4.0 * L)
    c["fi_re"], c["fi_im"], c["fi_imn"] = bf(fi.real), bf(fi.imag), bf(-fi.imag)

    # reversal stage-B variants: Zrev[k2'] uses G columns 63-k2' (main) and
    # (64-k2')%64 (the k1=0 column)
    def blockdiag(m):
        out = np.zeros((128, 128), np.complex128)
        out[:64, :64] = m
        out[64:, 64:] = m
        return out
    n2v = np.arange(64)[:, None]
    k2v = np.arange(64)[None, :]
    # rev as a partition permutation applied to the drained spectrum:
    # revZ[(s,k2'),k1'] = Z[(s,63-k2'),128-k1'] (k1'>=1; rev-read handles k1)
    # and revZ[(s,k2'),0] = Z[(s,(64-k2')%64),0]
    perm = np.zeros((64, 64))
    perm[63 - k2v[0], np.arange(64)] = 1.0
    perm0 = np.zeros((64, 64))
    perm0[(64 - np.arange(64)) % 64, np.arange(64)] = 1.0
    c["prev"] = bf(blockdiag(perm).real)
    c["prev0"] = bf(blockdiag(perm0).real)

    c["ident"] = bf(np.eye(128))
    c["ident_f32"] = f32(np.eye(128))

    p = np.arange(128)
    c["idxmat"] = f32(p[:, None] + 128.0 * np.arange(32)[None, :])
    c["pbc"] = f32(np.tile(p[None, :], (128, 1)))
    c["j128"] = f32(np.tile(128.0 * np.arange(32)[None, :], (128, 1)))
    return c


CONSTS = _host_consts()


def build_program(nc):
    # x arrives HOST-TRANSPOSED: [D, N] so the uv matmul needs no PE transpose
    x = nc.dram_tensor("x", [D, N], F32, kind="ExternalInput")
    wuv = nc.dram_tensor("wuv", [D, 2 * H], F32, kind="ExternalInput")
    wo = nc.dram_tensor("wo", [H, D], F32, kind="ExternalInput")
    rpeo = nc.dram_tensor("rpeo", [FEAT, H], F32, kind="ExternalInput")
    dg = nc.dram_tensor("dg", [1, H], F32, kind="ExternalInput")
    rpe_in = nc.dram_tensor("rpe_in", [1, FEAT], F32, kind="ExternalInput")
    rpe_hid = nc.dram_tensor("rpe_hid", [RPE_LAYERS, FEAT, FEAT], F32,
                             kind="ExternalInput")
    ln_g = nc.dram_tensor("ln_g", [RPE_LAYERS, FEAT], F32, kind="ExternalInput")
    ln_b = nc.dram_tensor("ln_b", [RPE_LAYERS, FEAT], F32, kind="ExternalInput")

    cds = {}
    for name, arr in CONSTS.items():
        dt = BF16 if arr.dtype == _NP_BF16 else F32
        cds[name] = nc.dram_tensor(name, list(arr.shape), dt, kind="ExternalInput")

    t_bf = nc.dram_tensor("t_bf", [H, N], BF16, kind="Internal")
    v_bf = nc.dram_tensor("v_bf", [H, N], BF16, kind="Internal")
    u_bf = nc.dram_tensor("u_bf", [H, N], BF16, kind="Internal")
    o_bf = nc.dram_tensor("o_bf", [H, N], BF16, kind="Internal")
    lng_dram = nc.dram_tensor("lng_dram", [1, H], F32, kind="Internal")
    out = nc.dram_tensor("out", [D, N], BF16, kind="ExternalOutput")

    ap = lambda t, off, pattern: bass.AP(tensor=t, offset=off, ap=pattern)

    with tile.TileContext(nc) as tc:
        with tc.tile_pool(name="consts", bufs=1) as cp:
            cs = {}
            for name, arr in CONSTS.items():
                dt = BF16 if arr.dtype == _NP_BF16 else F32
                ct = cp.tile(list(arr.shape), dt, tag=f"c_{name}")
                nc.sync.dma_start(out=ct, in_=cds[name][:, :])
                cs[name] = ct
            w_in_bc = cp.tile([128, FEAT], F32, tag="w_in_bc")
            nc.sync.dma_start(out=w_in_bc, in_=ap(rpe_in, 0, [[0, 128], [1, FEAT]]))
            lng_bc, lnb_bc = [], []
            for l in range(RPE_LAYERS):
                g_t = cp.tile([128, FEAT], F32, tag=f"lng{l}")
                b_t = cp.tile([128, FEAT], F32, tag=f"lnb{l}")
                nc.sync.dma_start(out=g_t, in_=ap(ln_g, l * FEAT, [[0, 128], [1, FEAT]]))
                nc.sync.dma_start(out=b_t, in_=ap(ln_b, l * FEAT, [[0, 128], [1, FEAT]]))
                lng_bc.append(g_t)
                lnb_bc.append(b_t)

            whid = []
            for l in range(RPE_LAYERS):
                wt4 = cp.tile([128, 128], F32, tag=f"whidf{l}")
                nc.vector.memset(wt4, 0.0)
                for j in range(4):
                    nc.sync.dma_start(
                        out=wt4[32 * j:32 * j + 32, 32 * j:32 * j + 32],
                        in_=rpe_hid[l, :, :])
                wb = cp.tile([128, 128], BF16, tag=f"whid{l}")
                nc.vector.tensor_copy(out=wb, in_=wt4)
                whid.append(wb)
            lng_bf, lnb_bf = [], []
            for l in range(RPE_LAYERS):
                gb_t = cp.tile([128, FEAT], BF16, tag=f"lngb{l}")
                bb_t = cp.tile([128, FEAT], BF16, tag=f"lnbb{l}")
                nc.vector.tensor_copy(out=gb_t, in_=lng_bc[l])
                nc.vector.tensor_copy(out=bb_t, in_=lnb_bc[l])
                lng_bf.append(gb_t)
                lnb_bf.append(bb_t)

            eps_t = cp.tile([128, 1], F32, tag="eps_t")
            nc.vector.memset(eps_t, LN_EPS)
            rpeo_f = cp.tile([128, H], F32, tag="rpeo_f")
            for j in range(4):
                nc.sync.dma_start(out=rpeo_f[32 * j:32 * j + 32, :], in_=rpeo[:, :])
            rpeo_sb = cp.tile([128, H], BF16, tag="rpeo_sb")
            nc.vector.tensor_copy(out=rpeo_sb, in_=rpeo_f)

            # decay -> lngam_col [128, 4]  (lngam_col[cp, cb] = ln gamma_{128cb+cp})
            with tc.tile_pool(name="dk", bufs=1) as dk:
                dg_sb = dk.tile([1, H], F32, tag="dg")
                nc.sync.dma_start(out=dg_sb, in_=dg[:, :])
                sig = dk.tile([1, H], F32, tag="sig")
                nc.scalar.activation(sig, dg_sb, AF.Sigmoid)
                gam = dk.tile([1, H], F32, tag="gam")
                nc.vector.tensor_scalar(gam, sig, 1.0 - LOWER, LOWER,
                                        ALU.mult, ALU.add)
                lngr = dk.tile([1, H], F32, tag="lngr")
                nc.scalar.activation(lngr, gam, AF.Ln)
                nc.sync.dma_start(out=lng_dram[:, :], in_=lngr)
            lngam_col = cp.tile([128, 4], F32, tag="lngam_col")
            nc.sync.dma_start(out=lngam_col,
                              in_=ap(lng_dram, 0, [[1, 128], [128, 4]]))

            # decay matrices: d0[cb][ch, s] = gamma_ch^s; rj[cb][ch, j] =
            # gamma_ch^(128 j) -- replaces a scalar Exp per (j, cb) block
            d0s, rjs = [], []
            for cb in range(4):
                d0 = cp.tile([128, 128], BF16, tag=f"d0_{cb}")
                nc.scalar.activation(d0, cs["pbc"], AF.Exp,
                                     scale=lngam_col[:, cb:cb + 1])
                rj = cp.tile([128, 32], F32, tag=f"rj_{cb}")
                nc.scalar.activation(rj, cs["j128"], AF.Exp,
                                     scale=lngam_col[:, cb:cb + 1])
                d0s.append(d0)
                rjs.append(rj)

            # =====================================================
            # Phase A: RPE MLP -> t_bf (channel-major bf16).
            # All 32 position-blocks processed at once: h_sm [128p, 32j, 32f]
            # (n = p + 128j). Wide DVE/ACT ops; transposes batched per layer.
            # =====================================================
            idxm = cs["idxmat"]
            with tc.tile_pool(name="rpe", bufs=2) as rp, \
                 tc.tile_pool(name="rpe_ps", bufs=2, space="PSUM") as rps:
                h_sm = rp.tile([128, 32, FEAT], BF16, tag="h_sm")
                for j in range(32):
                    nc.scalar.activation(h_sm[:, j, :], w_in_bc, AF.Silu,
                                         scale=idxm[:, j:j + 1])
                h_fm = rp.tile([128, 8, 128], BF16, tag="h_fm")
                for l in range(RPE_LAYERS):
                    mu = rp.tile([128, 32], F32, tag="mu")
                    nc.vector.tensor_reduce(mu, h_sm, AXX, ALU.add)
                    nc.vector.tensor_scalar_mul(mu, mu, 1.0 / FEAT)
                    hc = rp.tile([128, 32, FEAT], BF16, tag="hc")
                    nc.vector.tensor_tensor(
                        hc, h_sm, mu[:, :, None].to_broadcast((128, 32, FEAT)),
                        ALU.subtract)
                    sq = rp.tile([128, 32, FEAT], BF16, tag="sq")
                    nc.vector.tensor_tensor(sq, hc, hc, ALU.mult)
                    var = rp.tile([128, 32], F32, tag="var")
                    nc.vector.tensor_reduce(var, sq, AXX, ALU.add)
                    rstd = rp.tile([128, 32], F32, tag="rstd")
                    nc.scalar.activation(rstd, var, AF.Sqrt,
                                         scale=1.0 / FEAT, bias=eps_t)
                    nc.vector.reciprocal(rstd, rstd)
                    hn = rp.tile([128, 32, FEAT], BF16, tag="hn")
                    nc.vector.tensor_tensor(
                        hn, hc, rstd[:, :, None].to_broadcast((128, 32, FEAT)),
                        ALU.mult)
                    gb_ = lng_bf[l][:, None, :].to_broadcast((128, 32, FEAT))
                    bb_ = lnb_bf[l][:, None, :].to_broadcast((128, 32, FEAT))
                    hs = rp.tile([128, 32, FEAT], BF16, tag="hs")
                    nc.vector.tensor_tensor(hs, hn, gb_, ALU.mult)
                    nc.vector.tensor_tensor(hs, hs, bb_, ALU.add)
                    hs_flat = hs.rearrange("p a b -> p (a b)")
                    ln_fm = rp.tile([128, 8, 128], BF16, tag="ln_fm")
                    for tb in range(8):
                        pt = rps.tile([128, 128], BF16, tag="ptA")
                        nc.tensor.transpose(
                            pt, hs_flat[:, 128 * tb:128 * tb + 128], cs["ident"])
                        nc.vector.tensor_copy(out=ln_fm[:, tb, :], in_=pt)
                    for tb in range(8):
                        hp = rps.tile([128, 128], F32, tag="hpA")
                        nc.tensor.matmul(hp, whid[l], ln_fm[:, tb, :],
                                         start=True, stop=True)
                        nc.scalar.activation(h_fm[:, tb, :], hp, AF.Silu)
                    if l < RPE_LAYERS - 1:
                        h_sm = rp.tile([128, 32, FEAT], BF16, tag="h_sm")
                        hsm_flat = h_sm.rearrange("p a b -> p (a b)")
                        for tb in range(8):
                            pt2 = rps.tile([128, 128], BF16, tag="ptA")
                            nc.tensor.transpose(pt2, h_fm[:, tb, :], cs["ident"])
                            nc.vector.tensor_copy(
                                out=hsm_flat[:, 128 * tb:128 * tb + 128], in_=pt2)
                # output projection (32 feat -> 512 ch) + decay, 512 pos/mm
                for jj in range(4):
                    for cb in range(4):
                        for half in range(2):
                            to = rps.tile([128, 512], F32, tag="toA")
                            nc.tensor.matmul(
                                to,
                                rpeo_sb[32 * jj:32 * jj + 32,
                                        128 * cb:128 * cb + 128],
                                h_fm[32 * jj:32 * jj + 32,
                                     4 * half:4 * half + 4, :],
                                start=True, stop=True,
                                tile_position=(32 * jj, 0))
                            # drain with per-partition scale gamma^(128j)
                            tsb = rp.tile([128, 4, 128], BF16, tag="tsbA")
                            for g4 in range(4):
                                j = 4 * (4 * half + g4) + jj
                                nc.scalar.activation(
                                    tsb[:, g4, :],
                                    to[:, 128 * g4:128 * g4 + 128],
                                    AF.Copy, scale=rjs[cb][:, j:j + 1])
                            tt = rp.tile([128, 4, 128], BF16, tag="t_out")
                            nc.vector.tensor_tensor(
                                tt, tsb,
                                d0s[cb][:, None, :].to_broadcast((128, 4, 128)),
                                ALU.mult)
                            nc.sync.dma_start(
                                out=ap(t_bf,
                                       128 * cb * N + 128 * jj + 2048 * half,
                                       [[N, 128], [512, 4], [1, 128]]),
                                in_=tt)

            # =====================================================
            # Phase B: uv projection (f32r) + silu -> u_bf, v_bf
            # =====================================================
            with tc.tile_pool(name="pb_w", bufs=1) as wbp, \
                 tc.tile_pool(name="pb", bufs=2) as pb, \
                 tc.tile_pool(name="pb_ps", bufs=2, space="PSUM") as pps:
                wuv_f = wbp.tile([128, 8, 2 * H], F32, tag="wuv_f")
                nc.gpsimd.dma_start(
                    out=wuv_f,
                    in_=ap(wuv, 0, [[2 * H, 128], [128 * 2 * H, 8], [1, 2 * H]]))
                wuv_sb = wbp.tile([128, 8, 2 * H], BF16, tag="wuv_sb")
                nc.vector.tensor_copy(out=wuv_sb, in_=wuv_f)
                xt_all = wbp.tile([128, 8, N], BF16, tag="xt_all")
                for k in range(8):
                    xf = pb.tile([128, N], F32, tag="xf")
                    nc.sync.dma_start(out=xf, in_=x[128 * k:128 * k + 128, :])
                    nc.vector.tensor_copy(out=xt_all[:, k, :], in_=xf)
                for cblk in (4, 5, 6, 7, 0, 1, 2, 3):
                    for stg in range(8):
                        pu = pps.tile([128, 512], F32, tag="uv_mm")
                        for k in range(8):
                            nc.tensor.matmul(
                                pu, wuv_sb[:, k, 128 * cblk:128 * cblk + 128],
                                xt_all[:, k, 512 * stg:512 * stg + 512],
                                start=(k == 0), stop=(k == 7))
                        uv_sb = pb.tile([128, 512], BF16, tag="uv_out")
                        nc.scalar.activation(uv_sb, pu, AF.Silu)
                        dst = u_bf if cblk < 4 else v_bf
                        cbase = (cblk % 4) * 128
                        nc.sync.dma_start(
                            out=ap(dst, cbase * N + 512 * stg,
                                   [[N, 128], [1, 512]]),
                            in_=uv_sb)

            # =====================================================
            # Phase C: FFT conv, 4-stage software pipeline over quads
            # (16 channels each).  t occupies free cols 0:512, v 512:1024.
            #   f1: gather DMA + stage A mm + drain         (PSUM q0-q3)
            #   f2: twiddle + stage B straight mm + drains
            #       + rev mm + rev drains                   (PSUM z0-z3/q0-q3)
            #   b1: S/D combos + spectral product           (DVE/GpSimd only)
            #   b2: inverse gi + twiddle + transpose + fi   (PSUM z0-z3 + q0)
            # Emission order per iteration: b2(i-3) b1(i-2) f2(i-1) f1(i) so
            # every engine starts each iteration with ready work.
            # =====================================================
            with tc.tile_pool(name="pc", bufs=2) as pc, \
                 tc.tile_pool(name="pc_sp", bufs=2) as sp, \
                 tc.tile_pool(name="pc_ps", bufs=1, space="PSUM") as ps:

                def mm(psum_out, lhsT, rhs, start, stop):
                    nc.tensor.matmul(psum_out, lhsT, rhs, start=start, stop=stop,
                                     skip_group_check=True)

                def f1(q4):
                    """gather + stage A -> asb (interleaved re|im)."""
                    rr = pc.tile([64, 1024], BF16, tag="rr", name="rr")
                    ri = pc.tile([64, 1024], BF16, tag="ri", name="ri")
                    base = 16 * q4 * N
                    pat = [[64, 64], [4 * N, 4], [2 * N, 2], [1, 64]]
                    nc.sync.dma_start(out=rr[:, 0:512], in_=ap(t_bf, base, pat))
                    nc.sync.dma_start(out=ri[:, 0:512], in_=ap(t_bf, base + N, pat))
                    nc.sync.dma_start(out=rr[:, 512:1024], in_=ap(v_bf, base, pat))
                    nc.sync.dma_start(out=ri[:, 512:1024], in_=ap(v_bf, base + N, pat))

                    pa = [ps.tile([128, 512], F32, tag=f"q{i}", name=f"pa{i}")
                          for i in range(4)]
                    for c in range(8):
                        dst = pa[c // 2][:, 256 * (c % 2):256 * (c % 2) + 256]
                        mm(dst, rr[:, 128 * c:128 * c + 128], cs["fa_pa"], True, False)
                        mm(dst, ri[:, 128 * c:128 * c + 128], cs["fa_pb"], False, True)
                    asb = pc.tile([128, 2048], BF16, tag="asb", name="asb")
                    for i in range(4):
                        nc.scalar.activation(asb[:, 512 * i:512 * i + 512],
                                             pa[i], AF.Copy)
                    return dict(asb=asb, base=base, pat=pat)

                def f2(st):
                    """twiddle + de-interleave + stage B straight & rev."""
                    asb = st["asb"]
                    m1p = pc.tile([128, 2048], BF16, tag="m1p", name="m1p")
                    m2p = pc.tile([128, 2048], BF16, tag="m2p", name="m2p")
                    bt_re = pc.tile([128, 1024], BF16, tag="btre", name="btre")
                    bt_im = pc.tile([128, 1024], BF16, tag="btim", name="btim")
                    nc.vector.tensor_tensor(m1p, asb, cs["twt_ra"], ALU.mult)
                    nc.vector.tensor_tensor(m2p, asb, cs["twt_rb"], ALU.mult)
                    po_ = m1p.ap[0][0]
                    re_sl = lambda t_: ap(t_.tensor, t_.offset,
                                          [[po_, 128], [256, 8], [1, 128]])
                    im_sl = lambda t_: ap(t_.tensor, t_.offset + 128,
                                          [[po_, 128], [256, 8], [1, 128]])
                    bt_re3 = bt_re[:, bass.ts(0, 1024)].rearrange(
                        "p (c k) -> p c k", c=8)
                    bt_im3 = bt_im[:, bass.ts(0, 1024)].rearrange(
                        "p (c k) -> p c k", c=8)
                    nc.vector.tensor_tensor(bt_re3, re_sl(m1p), im_sl(m1p),
                                            ALU.subtract)
                    nc.vector.tensor_tensor(bt_im3, re_sl(m2p), im_sl(m2p),
                                            ALU.add)

                    pz = [ps.tile([128, 512], F32, tag=f"z{i}", name=f"pz{i}")
                          for i in range(4)]
                    # pz[0]=T_re pz[1]=Z_re pz[2]=T_im pz[3]=Z_im
                    bre_t, bre_v = bt_re[:, 0:512], bt_re[:, 512:1024]
                    bim_t, bim_v = bt_im[:, 0:512], bt_im[:, 512:1024]
                    mm(pz[2], cs["g_im"], bre_t, True, False)
                    mm(pz[3], cs["g_im"], bre_v, True, False)
                    mm(pz[0], cs["g_re"], bre_t, True, False)
                    mm(pz[1], cs["g_re"], bre_v, True, False)
                    mm(pz[2], cs["g_re"], bim_t, False, True)
                    mm(pz[3], cs["g_re"], bim_v, False, True)
                    mm(pz[0], cs["g_imn"], bim_t, False, True)
                    mm(pz[1], cs["g_imn"], bim_v, False, True)
                    tt_re = sp.tile([128, 1024], BF16, tag="tt_re", name="tt_re")
                    tt_im = sp.tile([128, 1024], BF16, tag="tt_im", name="tt_im")
                    vz_re = sp.tile([128, 1024], BF16, tag="vz_re", name="vz_re")
                    vz_im = sp.tile([128, 1024], BF16, tag="vz_im", name="vz_im")
                    nc.scalar.activation(tt_re[:, 0:512], pz[0], AF.Copy)
                    nc.scalar.activation(vz_re[:, 0:512], pz[1], AF.Copy)
                    nc.scalar.activation(tt_im[:, 0:512], pz[2], AF.Copy)
                    nc.scalar.activation(vz_im[:, 0:512], pz[3], AF.Copy)

                    def rev_rhs(t_):
                        return ap(t_.tensor, t_.offset + 127,
                                  [[t_.ap[0][0], 128], [128, 4], [-1, 127]])

                    def col0_rhs(t_):
                        return ap(t_.tensor, t_.offset,
                                  [[t_.ap[0][0], 128], [128, 4]])

                    pr = [ps.tile([128, 512], F32, tag=f"q{i}", name=f"pr{i}")
                          for i in range(4)]
                    # pr[0]=Tr_re pr[1]=Zr_re pr[2]=Tr_im pr[3]=Zr_im
                    def rslice(t_, a, b):
                        return t_[:, bass.ts(0, 512)].rearrange(
                            "p (q k) -> p q k", q=4)[:, :, a:b]
                    mm(rslice(pr[0], 1, 128), cs["prev"], rev_rhs(tt_re[:, 0:512]), True, True)
                    mm(rslice(pr[1], 1, 128), cs["prev"], rev_rhs(vz_re[:, 0:512]), True, True)
                    mm(rslice(pr[2], 1, 128), cs["prev"], rev_rhs(tt_im[:, 0:512]), True, True)
                    mm(rslice(pr[3], 1, 128), cs["prev"], rev_rhs(vz_im[:, 0:512]), True, True)
                    mm(rslice(pr[0], 0, 1), cs["prev0"], col0_rhs(tt_re[:, 0:512]), True, True)
                    mm(rslice(pr[1], 0, 1), cs["prev0"], col0_rhs(vz_re[:, 0:512]), True, True)
                    mm(rslice(pr[2], 0, 1), cs["prev0"], col0_rhs(tt_im[:, 0:512]), True, True)
                    mm(rslice(pr[3], 0, 1), cs["prev0"], col0_rhs(vz_im[:, 0:512]), True, True)
                    nc.scalar.activation(tt_re[:, 512:1024], pr[0], AF.Copy)
                    nc.scalar.activation(vz_re[:, 512:1024], pr[1], AF.Copy)
                    nc.scalar.activation(tt_im[:, 512:1024], pr[2], AF.Copy)
                    nc.scalar.activation(vz_im[:, 512:1024], pr[3], AF.Copy)
                    st.update(tt_re=tt_re, tt_im=tt_im, vz_re=vz_re, vz_im=vz_im)

                def b1(st):
                    """S/D combos (P/P2/M2 basis) + spectral product.
                    s4re = P-M2, d4re = P+M2, s4im = P2-M2, d4imn = -(d4im)."""
                    tt_re, tt_im = st["tt_re"], st["tt_im"]
                    vz_re, vz_im = st["vz_re"], st["vz_im"]
                    tre_s, tre_r = tt_re[:, 0:512], tt_re[:, 512:1024]
                    tim_s, tim_r = tt_im[:, 0:512], tt_im[:, 512:1024]
                    P = sp.tile([128, 512], BF16, tag="Pc", name="Pc")
                    P2 = sp.tile([128, 512], BF16, tag="P2c", name="P2c")
                    M2 = sp.tile([128, 512], BF16, tag="M2c", name="M2c")
                    nc.vector.tensor_tensor(P, tre_s, tre_r, ALU.add)
                    nc.vector.tensor_tensor(P2, tre_r, tre_s, ALU.subtract)
                    nc.vector.tensor_tensor(M2, tim_r, tim_s, ALU.subtract)
                    s4re = sp.tile([128, 512], BF16, tag="s4re", name="s4re")
                    s4im = sp.tile([128, 512], BF16, tag="s4im", name="s4im")
                    d4re = sp.tile([128, 512], BF16, tag="d4re", name="d4re")
                    d4imn = sp.tile([128, 512], BF16, tag="d4imn", name="d4imn")
                    nc.vector.tensor_tensor(s4re, P, M2, ALU.subtract)
                    nc.vector.tensor_tensor(s4im, P2, M2, ALU.subtract)
                    nc.gpsimd.tensor_tensor(d4re, P, M2, ALU.add)
                    nc.gpsimd.tensor_tensor(d4imn, P2, M2, ALU.add)

                    zv_re, zv_im = vz_re[:, 0:512], vz_im[:, 0:512]
                    zr_re, zr_im = vz_re[:, 512:1024], vz_im[:, 512:1024]
                    q_re = sp.tile([128, 512], BF16, tag="q_re", name="q_re")
                    q_im = sp.tile([128, 512], BF16, tag="q_im", name="q_im")
                    a1 = pc.tile([128, 512], BF16, tag="qa1", name="qa1")
                    a2 = pc.tile([128, 512], BF16, tag="qa2", name="qa2")
                    a3 = pc.tile([128, 512], BF16, tag="qa3", name="qa3")
                    a4 = pc.tile([128, 512], BF16, tag="qa4", name="qa4")
                    nc.vector.tensor_tensor(a1, zv_re, s4re, ALU.mult)
                    nc.vector.tensor_tensor(a2, zv_im, s4im, ALU.mult)
                    nc.vector.tensor_tensor(a3, zr_re, d4re, ALU.mult)
                    nc.vector.tensor_tensor(a4, zr_im, d4imn, ALU.mult)
                    nc.vector.tensor_tensor(a1, a1, a2, ALU.subtract)
                    nc.vector.tensor_tensor(a3, a3, a4, ALU.subtract)
                    nc.vector.tensor_tensor(q_re, a1, a3, ALU.add)
                    b1_ = pc.tile([128, 512], BF16, tag="qb1", name="qb1")
                    b2_ = pc.tile([128, 512], BF16, tag="qb2", name="qb2")
                    b3_ = pc.tile([128, 512], BF16, tag="qb3", name="qb3")
                    b4_ = pc.tile([128, 512], BF16, tag="qb4", name="qb4")
                    nc.gpsimd.tensor_tensor(b1_, zv_re, s4im, ALU.mult)
                    nc.gpsimd.tensor_tensor(b2_, zv_im, s4re, ALU.mult)
                    nc.gpsimd.tensor_tensor(b1_, b1_, b2_, ALU.add)
                    nc.vector.tensor_tensor(b3_, zr_re, d4imn, ALU.mult)
                    nc.vector.tensor_tensor(b4_, zr_im, d4re, ALU.mult)
                    nc.vector.tensor_tensor(b3_, b3_, b4_, ALU.add)
                    nc.vector.tensor_tensor(q_im, b1_, b3_, ALU.subtract)
                    st.update(q_re=q_re, q_im=q_im)

                def b2(st):
                    """inverse: gi + inv-twiddle + transpose + fi + out DMA."""
                    q_re, q_im = st["q_re"], st["q_im"]
                    pc_re = ps.tile([128, 512], F32, tag="z0", name="pc_re")
                    pc_im = ps.tile([128, 512], F32, tag="z1", name="pc_im")
                    mm(pc_im, cs["gi_im"], q_re, True, False)
                    mm(pc_re, cs["gi_re"], q_re, True, False)
                    mm(pc_im, cs["gi_re"], q_im, False, True)
                    mm(pc_re, cs["gi_imn"], q_im, False, True)
                    csb_re = pc.tile([128, 512], BF16, tag="csb_re", name="csb_re")
                    csb_im = pc.tile([128, 512], BF16, tag="csb_im", name="csb_im")
                    nc.scalar.activation(csb_re, pc_re, AF.Copy)
                    nc.scalar.activation(csb_im, pc_im, AF.Copy)
                    i1 = pc.tile([128, 512], BF16, tag="i1", name="i1")
                    i2 = pc.tile([128, 512], BF16, tag="i2", name="i2")
                    i3 = pc.tile([128, 512], BF16, tag="i3", name="i3")
                    i4 = pc.tile([128, 512], BF16, tag="i4", name="i4")
                    ct_re = pc.tile([128, 512], BF16, tag="ct_re", name="ct_re")
                    ct_im = pc.tile([128, 512], BF16, tag="ct_im", name="ct_im")
                    nc.vector.tensor_tensor(i1, csb_re, cs["ti_re"], ALU.mult)
                    nc.vector.tensor_tensor(i2, csb_im, cs["ti_im"], ALU.mult)
                    nc.vector.tensor_tensor(i3, csb_re, cs["ti_im"], ALU.mult)
                    nc.vector.tensor_tensor(i4, csb_im, cs["ti_re"], ALU.mult)
                    nc.gpsimd.tensor_tensor(ct_re, i1, i2, ALU.subtract)
                    nc.gpsimd.tensor_tensor(ct_im, i3, i4, ALU.add)
                    tp2 = ps.tile([128, 1024], BF16, tag="z2", name="tp2")
                    for tau in range(4):
                        nc.tensor.transpose(
                            tp2[:, 128 * tau:128 * tau + 128],
                            ct_re[:, 128 * tau:128 * tau + 128], cs["ident"])
                        nc.tensor.transpose(
                            tp2[:, 512 + 128 * tau:512 + 128 * tau + 128],
                            ct_im[:, 128 * tau:128 * tau + 128], cs["ident"])
                    ctt_re = pc.tile([128, 512], BF16, tag="ctt_re", name="ctt_re")
                    ctt_im = pc.tile([128, 512], BF16, tag="ctt_im", name="ctt_im")
                    nc.scalar.activation(ctt_re, tp2[:, 0:512], AF.Copy)
                    nc.scalar.activation(ctt_im, tp2[:, 512:1024], AF.Copy)
                    pO_re = ps.tile([64, 512], F32, tag="z3", name="pO_re")
                    pO_im = ps.tile([64, 512], F32, tag="q0", name="pO_im")
                    mm(pO_im, cs["fi_im"], ctt_re, True, False)
                    mm(pO_re, cs["fi_re"], ctt_re, True, False)
                    mm(pO_im, cs["fi_re"], ctt_im, False, True)
                    mm(pO_re, cs["fi_imn"], ctt_im, False, True)
                    o_re = pc.tile([64, 512], BF16, tag="o_re", name="o_re")
                    o_im = pc.tile([64, 512], BF16, tag="o_im", name="o_im")
                    nc.scalar.activation(o_re, pO_re, AF.Copy)
                    nc.scalar.activation(o_im, pO_im, AF.Copy)
                    nc.sync.dma_start(out=ap(o_bf, st["base"], st["pat"]), in_=o_re)
                    nc.sync.dma_start(out=ap(o_bf, st["base"] + N, st["pat"]), in_=o_im)

                sts = {}
                for i in range(NQUAD + 3):
                    if i >= 3:
                        b2(sts.pop(i - 3))
                    if i >= 2:
                        b1(sts[i - 2])
                    if i >= 1:
                        f2(sts[i - 1])
                    if i < NQUAD:
                        sts[i] = f1(i)

            # =====================================================
            # Phase D: gate + output projection (f32r partials)
            # =====================================================
            with tc.tile_pool(name="pd_w", bufs=1) as wdp, \
                 tc.tile_pool(name="pd", bufs=2) as pd, \
                 tc.tile_pool(name="pd_ps", bufs=2, space="PSUM") as dps:
                wo_f = wdp.tile([128, 4, D], F32, tag="wo_f")
                nc.gpsimd.dma_start(
                    out=wo_f, in_=ap(wo, 0, [[D, 128], [128 * D, 4], [1, D]]))
                wo_sb = wdp.tile([128, 4, D], BF16, tag="wo_sb")
                nc.vector.tensor_copy(out=wo_sb, in_=wo_f)
                for sb in range(8):
                    gts = []
                    for cb in range(4):
                        ut = pd.tile([128, 512], BF16, tag=f"g_u{cb}")
                        ot = pd.tile([128, 512], BF16, tag=f"g_o{cb}")
                        nc.sync.dma_start(
                            out=ut, in_=ap(u_bf, 128 * cb * N + 512 * sb,
                                           [[N, 128], [1, 512]]))
                        nc.sync.dma_start(
                            out=ot, in_=ap(o_bf, 128 * cb * N + 512 * sb,
                                           [[N, 128], [1, 512]]))
                        gt = pd.tile([128, 512], BF16, tag=f"g_g{cb}")
                        nc.vector.tensor_tensor(gt, ut, ot, ALU.mult)
                        gts.append(gt)
                    for ocblk in range(8):
                        po = dps.tile([128, 512], F32, tag="out_mm")
                        for cb in range(4):
                            nc.tensor.matmul(
                                po, wo_sb[:, cb, 128 * ocblk:128 * ocblk + 128],
                                gts[cb], start=(cb == 0), stop=(cb == 3))
                        os_ = pd.tile([128, 512], BF16, tag="out_sb")
                        nc.scalar.activation(os_, po, AF.Copy)
                        nc.sync.dma_start(
                            out=ap(out, 128 * ocblk * N + 512 * sb,
                                   [[N, 128], [1, 512]]),
                            in_=os_)
    return nc


_PROGRAM_CACHE = {}
LAST_RESULTS = []


def _get_program():
    if "nc" not in _PROGRAM_CACHE:
        nc = bacc.Bacc("TRN2", target_bir_lowering=False)
        build_program(nc)
        nc.compile()
        _PROGRAM_CACHE["nc"] = nc
    return _PROGRAM_CACHE["nc"]


def kernel(x, W_uv, W_o, rpe_in_w, rpe_hid_w, rpe_ln_g, rpe_ln_b, rpe_out_w,
           decay_gamma):
    x = np.asarray(x, np.float32)
    W_uv = np.asarray(W_uv, np.float32)
    W_o = np.asarray(W_o, np.float32)

    nc = _get_program()

    shared = dict(CONSTS)
    shared["rpe_in"] = np.ascontiguousarray(rpe_in_w, np.float32)
    shared["rpe_hid"] = np.ascontiguousarray(rpe_hid_w, np.float32)
    shared["ln_g"] = np.ascontiguousarray(rpe_ln_g, np.float32)
    shared["ln_b"] = np.ascontiguousarray(rpe_ln_b, np.float32)

    xT = [np.ascontiguousarray(x[b].T) for b in range(B)]
    in_maps = []
    for core in range(8):
        b, h = core // 2, core % 2
        c0 = h * H
        m = dict(shared)
        m["x"] = xT[b]
        m["wuv"] = np.ascontiguousarray(
            np.concatenate([W_uv[:, c0:c0 + H], W_uv[:, D1 + c0:D1 + c0 + H]],
                           axis=1))
        m["wo"] = np.ascontiguousarray(np.asarray(W_o, np.float32)[c0:c0 + H, :])
        m["rpeo"] = np.ascontiguousarray(np.asarray(rpe_out_w, np.float32)[:, c0:c0 + H])
        m["dg"] = np.ascontiguousarray(
            np.asarray(decay_gamma, np.float32)[None, c0:c0 + H])
        in_maps.append(m)

    trace = os.environ.get("KERNEL_TRACE", "0") == "1"
    tkw = {}
    if trace:
        tkw = dict(trace=True,
                   trace_cores=[int(c) for c in os.environ.get(
                       "KERNEL_TRACE_CORES", "0").split(",")])
    res = run_bass_kernel_spmd(nc, in_maps, core_ids=list(range(8)), **tkw)
    LAST_RESULTS.append(res)
    outs = [np.asarray(r["out"], np.float32) for r in res.results]
    final = np.empty((B, N, D), np.float32)
    for b in range(B):
        final[b] = (outs[2 * b] + outs[2 * b + 1]).T
    return final

